# revision 1
# baseline (speedup 1.0000x reference)
"""GAT (3-layer, PyG-style) on 8 Trainium2 NeuronCores via Bass/Tile.

Strategy (dst-sharded graph parallel):
  - Nodes sharded 8 ways by destination; edges partitioned by dst shard,
    sorted by dst, grouped into 128-dst "tiles" and 128-edge "chunks".
  - Per layer, a node table [N, row] holds [h(bf16) | a_src(f32) | a_dst(f32)]
    in HBM on every core (stitched on host between launches = halo exchange).
  - Per-edge source rows fetched with gpsimd dma_gather (int16 idx, low/high
    table-half split to fit int16).
  - a_dst[dst] expanded per-edge with a host-built one-hot S_T (fp8) matmul.
  - logits -> Prelu -> Exp on ACT; e * h[src] on DVE; segment-sum via
    host-built one-hot S (fp8) matmuls into PSUM, with e appended as extra
    rhs columns so segment softmax denominators come out of the same matmul.
  - Per-node epilogue: normalize, +bias, ELU, transpose, next-layer matmul
    (W pre-augmented with attention vectors so al_src/al_dst ride along).
"""
import numpy as np
import ml_dtypes
from contextlib import ExitStack

import concourse.bass as bass
import concourse.tile as tile
from concourse import bacc, mybir
from concourse import bass_utils
from concourse.masks import make_identity

P = 128
N_NODES = 50000
N_EDGES = 650000
NEG_SLOPE = 0.2
N_CORES = 8
NS = N_NODES // N_CORES            # 6250 nodes per shard
NT = (NS + P - 1) // P             # 49 dst tiles per core
NSP = NT * P                       # padded shard nodes (6272)
HALF = 32768                       # int16 gather limit -> low/high split
ROWW = 384                         # bf16 slots per table row (768B)
ROWW3 = 64                         # f32 per layer-3 table row (256B)
F = 256                            # feature width (H*C)
H = 4

FP8 = mybir.dt.float8e4
BF16 = mybir.dt.bfloat16
FP16 = mybir.dt.float16
F32 = mybir.dt.float32
I16 = mybir.dt.int16


# ----------------------------------------------------------------- host prep

def _wrap16(idx_flat):
    """[n] int array -> [128, n//16] int16 (16-partition wrap, replicated)."""
    n = len(idx_flat)
    a = np.asarray(idx_flat, dtype=np.int16).reshape(n // 16, 16).T
    return np.tile(a, (8, 1))


def build_schedule(src, dst):
    """Partition edges by dst shard / dst tile / src half, pad to chunks."""
    order = np.argsort(dst, kind="stable")
    src = src[order]
    dst = dst[order]
    core_of = dst // NS
    core_starts = np.searchsorted(core_of, np.arange(N_CORES + 1))

    per_core = []
    for c in range(N_CORES):
        s0, s1 = core_starts[c], core_starts[c + 1]
        cs, cd = src[s0:s1], dst[s0:s1] - c * NS
        tile_of = cd // P
        tile_starts = np.searchsorted(tile_of, np.arange(NT + 1))
        tiles = []
        for t in range(NT):
            t0, t1 = tile_starts[t], tile_starts[t + 1]
            ts_, td_ = cs[t0:t1], cd[t0:t1] - t * P
            lo = ts_ < HALF
            tiles.append(((ts_[lo], td_[lo]), (ts_[~lo] - HALF, td_[~lo])))
        per_core.append(tiles)

    LCH = np.zeros(NT, np.int64)
    HCH = np.zeros(NT, np.int64)
    for t in range(NT):
        for c in range(N_CORES):
            (ls, _), (hs, _) = per_core[c][t]
            LCH[t] = max(LCH[t], -(-len(ls) // P))
            HCH[t] = max(HCH[t], -(-len(hs) // P))
        LCH[t] = max(LCH[t], 1)
    NCH = LCH + HCH
    ch0 = np.concatenate([[0], np.cumsum(NCH)])
    TOTCH = int(ch0[-1])

    idx16 = np.zeros((N_CORES, P, TOTCH * 8), np.int16)
    S = np.zeros((N_CORES, P, TOTCH, P), ml_dtypes.float8_e4m3)
    ST = np.zeros((N_CORES, P, TOTCH, P), ml_dtypes.float8_e4m3)
    one = ml_dtypes.float8_e4m3(1.0)
    for c in range(N_CORES):
        for t in range(NT):
            for half, (es, ed) in enumerate(per_core[c][t]):
                nch = int((LCH[t], HCH[t])[half])
                if nch == 0:
                    continue
                base = int(ch0[t]) + (int(LCH[t]) if half else 0)
                n = nch * P
                e_pad = np.zeros(n, np.int64)
                e_pad[: len(es)] = es
                if len(es):
                    k = np.arange(len(es))
                    S[c, k % P, base + k // P, ed] = one
                    ST[c, ed, base + k // P, k % P] = one
                idx16[c, :, base * 8 : (base + nch) * 8] = _wrap16(e_pad)
    return dict(LCH=LCH, HCH=HCH, NCH=NCH, ch0=ch0, TOTCH=TOTCH,
                idx16=idx16, S=S, ST=ST)


# ------------------------------------------------------------- bass builders

class Prog:
    def __init__(self):
        self.nc = bacc.Bacc("TRN2", target_bir_lowering=False, debug=False,
                            num_devices=N_CORES)
        self.in_aps = {}
        self.out_aps = {}

    def inp(self, name, shape, dt):
        ap = self.nc.dram_tensor(name, list(shape), dt, kind="ExternalInput").ap()
        self.in_aps[name] = ap
        return ap

    def out(self, name, shape, dt):
        ap = self.nc.dram_tensor(name, list(shape), dt, kind="ExternalOutput").ap()
        self.out_aps[name] = ap
        return ap


def _table_write(nc, sb, h_ps, tabout, t):
    """PSUM [128, 264] -> table row tile (bf16 h + raw f32 as/ad) -> HBM."""
    trow = sb.tile([P, ROWW], BF16, tag="trow")
    nc.vector.memset(trow[:, F + 16 : ROWW], 0.0)
    nc.scalar.copy(trow[:, 0:F], h_ps[:, 0:F])
    nc.vector.tensor_copy(trow[:, F : F + 16].bitcast(F32), h_ps[:, F : F + 8])
    nc.sync.dma_start(tabout[t * P : (t + 1) * P, :], trow[:])


def build_launch_A():
    """x_shard @ Waug1 -> table1 rows [h1 | as1 | ad1]."""
    pr = Prog()
    nc = pr.nc
    x = pr.inp("x", [NSP, P], F32)
    w1 = pr.inp("w1", [P, F + 8], F32)
    tab = pr.out("tab", [NSP, ROWW], BF16)
    with tile.TileContext(nc) as tc, ExitStack() as ctx:
        sb = ctx.enter_context(tc.tile_pool(name="sb", bufs=5))
        ps = ctx.enter_context(tc.tile_pool(name="ps", bufs=4, space="PSUM"))
        cpool = ctx.enter_context(tc.tile_pool(name="cp", bufs=1))
        ident = cpool.tile([P, P], F32)
        make_identity(nc, ident[:])
        w1t = cpool.tile([P, F + 8], F32)
        nc.sync.dma_start(w1t[:], w1)
        B4 = 7  # tiles per batched x-load / table-store DMA
        for t0 in range(0, NT, B4):
            nb = min(B4, NT - t0)
            xt = sb.tile([P, B4, P], F32, tag="xt")
            nc.sync.dma_start(
                xt[:, 0:nb, :],
                x[t0 * P : (t0 + nb) * P, :].rearrange("(b p) f -> p b f", p=P))
            trow = sb.tile([P, B4, ROWW], BF16, tag="trow")
            nc.vector.memset(trow[:], 0.0)
            for j in range(nb):
                t = t0 + j
                xT_ps = ps.tile([P, P], F32, space="PSUM", tag="xT")
                nc.tensor.transpose(xT_ps[:], xt[:, j, :], ident[:])
                xT = sb.tile([P, P], F32, tag="xTs")
                nc.scalar.copy(xT[:], xT_ps[:])
                h_ps = ps.tile([P, F + 8], F32, space="PSUM", tag="hps")
                nc.tensor.matmul(h_ps[:], lhsT=xT[:], rhs=w1t[:],
                                 start=True, stop=True)
                nc.scalar.copy(trow[:, j, 0:F], h_ps[:, 0:F])
                nc.vector.tensor_copy(
                    trow[:, j, F : F + 16].bitcast(F32), h_ps[:, F : F + 8])
            nc.sync.dma_start(
                tab[t0 * P : (t0 + nb) * P, :].rearrange("(b p) f -> p b f", p=P),
                trow[:, 0:nb, :])
    nc.compile()
    return pr


def build_launch_agg(sch, layer, b3=0.0, stage=99):
    """layer=1: L1 agg -> table2 (bf16 rows); layer=2: L2 agg -> table3
    (f32 rows); layer=3: L3 agg -> output.  stage: debug early-exit level."""
    pr = Prog()
    nc = pr.nc
    TOTCH = sch["TOTCH"]
    last = layer == 3
    roww = ROWW3 if last else ROWW
    tab_dt = F32 if last else BF16
    table = pr.inp("table", [N_NODES, roww], tab_dt)
    mytab = pr.inp("mytab", [NSP, roww], tab_dt)
    idx16 = pr.inp("idx16", [P, TOTCH * 8], I16)
    S_in = pr.inp("S", [P, TOTCH, P], FP8)
    ST_in = pr.inp("ST", [P, TOTCH, P], FP8)
    if layer == 1:
        naug = F + 8
        waug = pr.inp("waug", [F, naug], F32)
        tabout = pr.out("tabout", [NSP, ROWW], BF16)
    elif layer == 2:
        naug = 3
        waug = pr.inp("waug", [F, naug], F32)
        tabout = pr.out("tabout", [NSP, ROWW3], F32)
    else:
        outv = pr.out("outv", [NSP, 1], F32)
    if not last:
        wcol = pr.inp("wcol", [1, naug], F32)
        bias = pr.inp("bias", [P, 2], F32)
        nbias = pr.inp("nbias", [P, 2], F32)

    NAGG = (F + 4) if not last else 2
    nad = H if not last else 1
    with tile.TileContext(nc) as tc, ExitStack() as ctx:
        sb = ctx.enter_context(tc.tile_pool(name="sb", bufs=3))
        sbg = ctx.enter_context(tc.tile_pool(name="sbg", bufs=3))
        ps = ctx.enter_context(tc.tile_pool(name="ps", bufs=2, space="PSUM"))
        psa = ctx.enter_context(tc.tile_pool(name="psa", bufs=2, space="PSUM"))
        cpool = ctx.enter_context(tc.tile_pool(name="cp", bufs=1))
        ident = cpool.tile([P, P], F32)
        make_identity(nc, ident[:])
        if not last:
            waug_t = cpool.tile([P, F // P, naug], F32, tag="waug")
            for k in range(F // P):
                nc.sync.dma_start(waug_t[:, k, :], waug[k * P : (k + 1) * P, :])
            wcol_t = cpool.tile([1, naug], F32, tag="wcol")
            nc.sync.dma_start(wcol_t[:], wcol)
            bias_t = cpool.tile([P, 2], F32, tag="bias")
            nc.sync.dma_start(bias_t[:], bias)
            nbias_t = cpool.tile([P, 2], F32, tag="nbias")
            nc.sync.dma_start(nbias_t[:], nbias)
            negone = cpool.tile([1, P], F32, tag="negone")
            nc.vector.memset(negone[:], -1.0)
        else:
            obuf = cpool.tile([P, NT], F32, tag="obuf")

        for t in range(NT):
            NCH = int(sch["NCH"][t])
            LCH = int(sch["LCH"][t])
            HCH = int(sch["HCH"][t])
            c0 = int(sch["ch0"][t])
            idx_t = sb.tile([P, NCH * 8], I16, tag="idx")
            nc.sync.dma_start(idx_t[:], idx16[:, c0 * 8 : (c0 + NCH) * 8])
            s_t = sb.tile([P, NCH, P], FP8, tag="S")
            nc.sync.dma_start(s_t[:], S_in[:, c0 : c0 + NCH, :])
            st_t = sb.tile([P, NCH, P], FP8, tag="ST")
            nc.sync.dma_start(st_t[:], ST_in[:, c0 : c0 + NCH, :])
            g_t = sbg.tile([P, NCH, roww], tab_dt, tag="G")
            GMAX = 8  # 1024-descriptor SWDGE ring cap per dma_gather
            for a0, a1, base in ((0, LCH, 0), (LCH, NCH, HALF)):
                for j0 in range(a0, a1, GMAX):
                    j1 = min(j0 + GMAX, a1)
                    nc.gpsimd.dma_gather(
                        out_ap=g_t[:, j0:j1, :],
                        in_ap=table if base == 0 else table[base:, :],
                        idxs_ap=idx_t[:, j0 * 8 : j1 * 8],
                        num_idxs=(j1 - j0) * P, num_idxs_reg=(j1 - j0) * P,
                        elem_size=roww)
            # adtile: own-shard a_dst rows for this tile, cast to fp16
            if not last:
                adraw = sb.tile([P, 16], BF16, tag="adraw")
                nc.sync.dma_start(adraw[:], mytab[t * P : (t + 1) * P, F : F + 16])
                ad_f32 = adraw[:].bitcast(F32)[:, 4:8]
            else:
                adraw = sb.tile([P, 4], F32, tag="adraw")
                nc.sync.dma_start(adraw[:], mytab[t * P : (t + 1) * P, 0:4])
                ad_f32 = adraw[:, 2:3]
            adt = sb.tile([P, nad], FP16, tag="adt")
            nc.vector.tensor_copy(adt[:], ad_f32)
            if stage == 0:
                trow = sb.tile([P, ROWW if not last else ROWW3], tab_dt, tag="trow")
                nc.vector.tensor_copy(trow[:], g_t[:, 0, :])
                nc.sync.dma_start(tabout[t * P : (t + 1) * P, :], trow[:])
                continue
            # a_dst expansion matmuls (per chunk) into one PSUM strip
            zps = ps.tile([P, NCH * nad], F32, space="PSUM", tag="zps")
            for j in range(NCH):
                nc.tensor.matmul(zps[:, j * nad : (j + 1) * nad],
                                 lhsT=st_t[:, j, :], rhs=adt[:],
                                 start=True, stop=True)
            # z = a_src + expanded a_dst ; e = exp(prelu(z))
            if not last:
                as_ap = g_t[:, :, F : F + 16].bitcast(F32)[:, :, 0:4]
            else:
                as_ap = g_t[:, :, 1:2]
            z_t = sb.tile([P, NCH, nad], F32, tag="z")
            nc.vector.tensor_tensor(
                out=z_t[:], in0=as_ap,
                in1=zps[:].rearrange("p (c h) -> p c h", h=nad),
                op=mybir.AluOpType.add)
            l_t = sb.tile([P, NCH, nad], F32, tag="l")
            nc.scalar.activation(l_t[:], z_t[:],
                                 mybir.ActivationFunctionType.Prelu,
                                 alpha=NEG_SLOPE)
            e_t = sb.tile([P, NCH, nad], F32, tag="e")
            nc.scalar.activation(e_t[:], l_t[:],
                                 mybir.ActivationFunctionType.Exp)
            if stage == 1:
                trow = sb.tile([P, ROWW if not last else ROWW3], tab_dt, tag="trow")
                nc.vector.memset(trow[:], 0.0)
                nc.vector.tensor_copy(trow[:, 0 : NCH * nad], e_t[:])
                nc.sync.dma_start(tabout[t * P : (t + 1) * P, :], trow[:])
                continue
            # weighted messages rhs = [e*h | e]
            eg_t = sbg.tile([P, NCH, NAGG], BF16, tag="eg")
            if not last:
                nc.vector.tensor_tensor(
                    out=eg_t[:, :, 0:F].rearrange("p c (h f) -> p c h f", h=H),
                    in0=g_t[:, :, 0:F].rearrange("p c (h f) -> p c h f", h=H),
                    in1=e_t[:].broadcast_to([P, NCH, H, F // H]),
                    op=mybir.AluOpType.mult)
                nc.vector.tensor_copy(eg_t[:, :, F : F + 4], e_t[:])
            else:
                nc.vector.tensor_tensor(
                    out=eg_t[:, :, 0:1], in0=g_t[:, :, 0:1], in1=e_t[:],
                    op=mybir.AluOpType.mult)
                nc.vector.tensor_copy(eg_t[:, :, 1:2], e_t[:])
            if stage == 2:
                trow = sb.tile([P, ROWW if not last else ROWW3], tab_dt, tag="trow")
                nc.vector.tensor_copy(trow[:, 0:NAGG], eg_t[:, 0, :])
                nc.vector.memset(trow[:, NAGG:], 0.0)
                nc.sync.dma_start(tabout[t * P : (t + 1) * P, :], trow[:])
                continue
            # aggregation matmuls
            agg = psa.tile([P, NAGG], F32, space="PSUM", tag="agg")
            for j in range(NCH):
                nc.tensor.matmul(agg[:], lhsT=s_t[:, j, :], rhs=eg_t[:, j, :],
                                 start=(j == 0), stop=(j == NCH - 1))
            if stage == 3:
                trow = sb.tile([P, ROWW if not last else ROWW3], tab_dt, tag="trow")
                nc.vector.tensor_copy(trow[:, 0:NAGG], agg[:])
                nc.vector.memset(trow[:, NAGG:], 0.0)
                nc.sync.dma_start(tabout[t * P : (t + 1) * P, :], trow[:])
                continue
            # epilogue
            if last:
                den = sb.tile([P, 1], F32, tag="den")
                nc.vector.tensor_scalar_add(den[:], agg[:, 1:2], 1e-16)
                r_t = sb.tile([P, 1], F32, tag="r")
                nc.vector.reciprocal(r_t[:], den[:])
                nc.vector.tensor_tensor(out=obuf[:, t : t + 1], in0=agg[:, 0:1],
                                        in1=r_t[:], op=mybir.AluOpType.mult)
                if b3 != 0.0:
                    nc.vector.tensor_scalar_add(
                        obuf[:, t : t + 1], obuf[:, t : t + 1], float(b3))
                continue
            den = sb.tile([P, H], F32, tag="den")
            nc.vector.tensor_scalar_add(den[:], agg[:, F : F + 4], 1e-16)
            r_t = sb.tile([P, H], F32, tag="r")
            nc.vector.reciprocal(r_t[:], den[:])
            xn = sb.tile([P, F], F32, tag="xn")
            for h in range(H):
                nc.scalar.mul(xn[:, h * 64 : (h + 1) * 64],
                              agg[:, h * 64 : (h + 1) * 64], r_t[:, h : h + 1])
            # ELU(x + b) = relu(z+b) + exp(min(z+b,0)) - 1, -1 folded into
            # matmul via negone row; done on transposed tiles (bias per part.)
            h_ps = psa.tile([P, naug], F32, space="PSUM", tag="hps")
            for k in range(2):
                xT_ps = ps.tile([P, P], F32, space="PSUM", tag="xT")
                nc.tensor.transpose(xT_ps[:], xn[:, k * P : (k + 1) * P], ident[:])
                p_t = sb.tile([P, P], F32, tag="p")
                nc.scalar.activation(p_t[:], xT_ps[:],
                                     mybir.ActivationFunctionType.Relu,
                                     bias=bias_t[:, k : k + 1])
                m_t = sb.tile([P, P], F32, tag="m")
                nc.scalar.activation(m_t[:], xT_ps[:],
                                     mybir.ActivationFunctionType.Relu,
                                     bias=nbias_t[:, k : k + 1], scale=-1.0)
                q_t = sb.tile([P, P], F32, tag="q")
                nc.scalar.activation(q_t[:], m_t[:],
                                     mybir.ActivationFunctionType.Exp,
                                     scale=-1.0)
                xe_t = sb.tile([P, P], F32, tag="xe")
                nc.vector.tensor_tensor(out=xe_t[:], in0=p_t[:], in1=q_t[:],
                                        op=mybir.AluOpType.add)
                nc.tensor.matmul(h_ps[:], lhsT=xe_t[:], rhs=waug_t[:, k, :],
                                 start=(k == 0), stop=False)
            nc.tensor.matmul(h_ps[:], lhsT=negone[:], rhs=wcol_t[:],
                             start=False, stop=True)
            if layer == 1:
                _table_write(nc, sb, h_ps, tabout, t)
            else:
                trow = sb.tile([P, ROWW3], F32, tag="trow")
                nc.vector.memset(trow[:], 0.0)
                nc.vector.tensor_copy(trow[:, 0:3], h_ps[:, 0:3])
                nc.sync.dma_start(tabout[t * P : (t + 1) * P, :], trow[:])
        if last:
            nc.sync.dma_start(
                outv.rearrange("(t p) o -> p t o", p=P).squeeze(-1), obuf[:])
    nc.compile()
    return pr


# --------------------------------------------------------------- the kernel

LAST_TIMES = {}


def _run(pr, in_maps, tag=None):
    if tag is not None:
        try:
            from concourse.timeline_sim import TimelineSim
            LAST_TIMES[tag] = TimelineSim(pr.nc, trace=False).simulate() / 1e9
        except Exception:
            pass
    res = bass_utils.run_bass_kernel_spmd(
        pr.nc, in_maps, core_ids=list(range(N_CORES)))
    return res.results


def _blockdiag_A(a_src, a_dst):
    Hh, C = a_src.shape
    A = np.zeros((Hh * C, 2 * Hh), np.float32)
    for h in range(Hh):
        A[h * C : (h + 1) * C, h] = a_src[h]
        A[h * C : (h + 1) * C, Hh + h] = a_dst[h]
    return A


def _pad_rows(a, n):
    out = np.zeros((n,) + a.shape[1:], a.dtype)
    out[: len(a)] = a
    return out


def kernel(x, edge_index, W1, a_src1, a_dst1, b1, W2, a_src2, a_dst2, b2,
           W3, a_src3, a_dst3, b3):
    x = np.asarray(x, np.float32)
    ei = np.asarray(edge_index)
    loops = np.arange(N_NODES, dtype=np.int64)
    src = np.concatenate([ei[0], loops]).astype(np.int64)
    dst = np.concatenate([ei[1], loops]).astype(np.int64)

    sch = build_schedule(src, dst)

    W1 = np.asarray(W1, np.float32); W2 = np.asarray(W2, np.float32)
    W3 = np.asarray(W3, np.float32)
    Waug1 = np.concatenate(
        [W1, W1 @ _blockdiag_A(np.asarray(a_src1), np.asarray(a_dst1))], 1)
    Waug2 = np.concatenate(
        [W2, W2 @ _blockdiag_A(np.asarray(a_src2), np.asarray(a_dst2))], 1)
    Waug3 = np.concatenate(
        [W3, W3 * float(np.asarray(a_src3)[0, 0]),
         W3 * float(np.asarray(a_dst3)[0, 0])], 1).astype(np.float32)
    wcol2 = Waug2.sum(0, keepdims=True).astype(np.float32)
    wcol3 = Waug3.sum(0, keepdims=True).astype(np.float32)
    b1T = np.asarray(b1, np.float32).reshape(2, P).T.copy()
    b2T = np.asarray(b2, np.float32).reshape(2, P).T.copy()

    # launch A: table1 from x
    prA = build_launch_A()
    inA = []
    for c in range(N_CORES):
        inA.append(dict(x=_pad_rows(x[c * NS : (c + 1) * NS], NSP), w1=Waug1))
    resA = _run(prA, inA, tag="A")
    tab1 = np.concatenate([resA[c]["tab"][:NS] for c in range(N_CORES)], 0)
    tab1 = np.ascontiguousarray(tab1)

    # launch B: L1 aggregation -> table2
    prB = build_launch_agg(sch, 1)
    inB = [dict(table=tab1, mytab=_pad_rows(tab1[c * NS : (c + 1) * NS], NSP),
                idx16=sch["idx16"][c], S=sch["S"][c], ST=sch["ST"][c],
                waug=Waug2, wcol=wcol2, bias=b1T, nbias=np.ascontiguousarray(-b1T))
           for c in range(N_CORES)]
    resB = _run(prB, inB, tag="B")
    tab2 = np.ascontiguousarray(
        np.concatenate([resB[c]["tabout"][:NS] for c in range(N_CORES)], 0))

    # launch C: L2 aggregation -> table3
    prC = build_launch_agg(sch, 2)
    inC = [dict(table=tab2, mytab=_pad_rows(tab2[c * NS : (c + 1) * NS], NSP),
                idx16=sch["idx16"][c], S=sch["S"][c], ST=sch["ST"][c],
                waug=Waug3, wcol=wcol3, bias=b2T, nbias=np.ascontiguousarray(-b2T))
           for c in range(N_CORES)]
    resC = _run(prC, inC, tag="C")
    tab3 = np.ascontiguousarray(
        np.concatenate([resC[c]["tabout"][:NS] for c in range(N_CORES)], 0))

    # launch D: L3 aggregation -> out
    prD = build_launch_agg(sch, 3, b3=float(np.asarray(b3).reshape(-1)[0]))
    inD = [dict(table=tab3, mytab=_pad_rows(tab3[c * NS : (c + 1) * NS], NSP),
                idx16=sch["idx16"][c], S=sch["S"][c], ST=sch["ST"][c])
           for c in range(N_CORES)]
    resD = _run(prD, inD, tag="D")
    out = np.concatenate([resD[c]["outv"][:NS] for c in range(N_CORES)], 0)
    return np.ascontiguousarray(out.astype(np.float32))



# revision 22
# speedup vs baseline: 1.1763x; 1.1763x over previous
"""GAT (3-layer, PyG-style) on 8 Trainium2 NeuronCores via Bass/Tile.

Strategy (dst-sharded graph parallel, v2):
  - Nodes permuted for per-tile load balance, sharded 8 ways by destination;
    edges partitioned by (core, dst tile), split lo/hi by source position
    (int16 gather limit), chunked to 128 edges.
  - Per layer a node table [N, 256] bf16 holds the head-features ROTATED so
    that slots 0,1 of each head are exactly al_src / al_dst (basis matrix M
    with columns [a_src | a_dst | orthonormal complement] folded into W).
    512B rows hit the DMA descriptor cost floor; the inverse rotation is one
    extra 128-col matmul per half in the epilogue.
  - Self-loops are a constant-identity chunk fed from the core's own rows
    (no gather, no one-hot bytes).
  - idx16 | S | ST packed into one blob DMA per 4-tile group; softmax is
    pre-normalized: denominators first (4-col matmuls), 1/den expanded per
    edge via the already-loaded ST, then alpha replaces e everywhere.
  - alpha replicated into bf16 pairs so the big alpha*h multiply runs in the
    DVE 2x_1p mode; epilogue matmuls run bf16/f32r at 1 cycle/column.
"""
import numpy as np
import ml_dtypes
from contextlib import ExitStack

import concourse.bass as bass
import concourse.tile as tile
from concourse import bacc, mybir
from concourse import bass_utils
from concourse.masks import make_identity

P = 128
N_NODES = 50000
N_EDGES = 650000
NEG_SLOPE = 0.2
N_CORES = 8
NS = N_NODES // N_CORES            # 6250 nodes per shard
NT = (NS + P - 1) // P             # 49 dst tiles per core
NSP = NT * P                       # padded shard nodes (6272)
HALF = 32768                       # int16 gather limit -> low/high split
ROW = 256                          # bf16 slots per table row (512B), layers 1,2
ROW3 = 128                         # bf16 slots per layer-3 table row (256B)
F = 256                            # feature width (H*C)
H = 4
GT = 4                             # tiles per group
GMAX = 8                           # chunks per dma_gather (1024-desc ucode cap)

FP8 = mybir.dt.float8e4
BF16 = mybir.dt.bfloat16
FP16 = mybir.dt.float16
F32 = mybir.dt.float32
F32R = mybir.dt.float32r
I16 = mybir.dt.int16
U8 = mybir.dt.uint8

SLOTS01 = [h * 64 + s for h in range(H) for s in range(2)]


# ----------------------------------------------------------------- host prep

def _wrap16(idx_flat):
    """[n] int array -> [128, n//16] int16 (16-partition wrap, replicated)."""
    n = len(idx_flat)
    a = np.asarray(idx_flat, dtype=np.int16).reshape(n // 16, 16).T
    return np.tile(a, (8, 1))


def _balance_perm(dst):
    """Permute nodes so per-(core,tile) edge counts are balanced.

    Returns pos_of_node [N] (permuted global position: core*NS + local)."""
    deg = np.bincount(dst, minlength=N_NODES)
    order = np.argsort(-deg, kind="stable")
    NB = N_CORES * NT                       # bins; bin b -> (tile b//8, core b%8)
    cap = np.empty(NB, np.int64)
    for t in range(NT):
        c = 128 if t < NT - 1 else NS - 128 * (NT - 1)
        cap[t * N_CORES:(t + 1) * N_CORES] = c
    # snake deal by descending degree over non-full bins
    snake = []
    for r in range(128):
        idxs = [b for b in range(NB) if cap[b] > r]
        if r % 2:
            idxs = idxs[::-1]
        snake.extend(idxs)
    assert len(snake) == N_NODES
    fill = np.zeros(NB, np.int64)
    pos_of_node = np.empty(N_NODES, np.int64)
    for i, n in enumerate(order):
        b = snake[i]
        t, c = b // N_CORES, b % N_CORES
        pos_of_node[n] = c * NS + t * 128 + fill[b]
        fill[b] += 1
    return pos_of_node


def build_schedule(src, dst, pos_of_node):
    """Edges by (core, tile, half); per-tile chunk counts maxed over cores;
    blob = [idx16 | S | ST] per 4-tile group."""
    spos = pos_of_node[src]
    dpos = pos_of_node[dst]
    core_of = dpos // NS
    local = dpos % NS
    tile_of = local // 128
    dslot = local % 128
    is_hi = spos >= HALF

    # per (core, tile): lo and hi edge arrays (srcidx, dslot)
    per = {}
    okey = core_of * (NT * 2) + tile_of * 2 + is_hi
    order = np.argsort(okey, kind="stable")
    so, do_, ko = spos[order], dslot[order], okey[order]
    bounds = np.searchsorted(ko, np.arange(N_CORES * NT * 2 + 1))
    cnt = np.zeros((N_CORES, NT, 2), np.int64)
    for c in range(N_CORES):
        for t in range(NT):
            for hh in range(2):
                k = c * (NT * 2) + t * 2 + hh
                b0, b1 = bounds[k], bounds[k + 1]
                per[(c, t, hh)] = (so[b0:b1] - (HALF if hh else 0), do_[b0:b1])
                cnt[c, t, hh] = b1 - b0

    LCH = np.maximum(-(-cnt[:, :, 0].max(0) // 128), 0)
    HCH = np.maximum(-(-cnt[:, :, 1].max(0) // 128), 0)

    # groups of GT tiles
    groups = []
    t0 = 0
    while t0 < NT:
        gt = min(GT, NT - t0)
        groups.append((t0, gt))
        t0 += gt

    # chunk layout per group: [tile-major lo][tile-major hi]
    meta = []
    TOTCH = int(LCH.sum() + HCH.sum())
    idx_all = np.zeros((N_CORES, P, TOTCH * 8), np.int16)
    one = ml_dtypes.float8_e4m3(1.0)
    S = np.zeros((N_CORES, P, TOTCH, P), ml_dtypes.float8_e4m3)
    ST = np.zeros((N_CORES, P, TOTCH, P), ml_dtypes.float8_e4m3)
    ch = 0
    for (t0, gt) in groups:
        lw = int(LCH[t0:t0 + gt].sum())
        hw = int(HCH[t0:t0 + gt].sum())
        tinfo = []
        lo_off = 0
        hi_off = 0
        for j in range(gt):
            t = t0 + j
            tinfo.append((int(lo_off), int(LCH[t]), int(hi_off), int(HCH[t])))
            lo_off += int(LCH[t])
            hi_off += int(HCH[t])
        meta.append(dict(t0=t0, gt=gt, lw=lw, hw=hw, ch0=ch, tinfo=tinfo))
        for c in range(N_CORES):
            for j in range(gt):
                t = t0 + j
                for hh in range(2):
                    nch = int((LCH[t], HCH[t])[hh])
                    if nch == 0:
                        continue
                    base = ch + (tinfo[j][0] if hh == 0 else lw + tinfo[j][2])
                    es, ed = per[(c, t, hh)]
                    n = nch * 128
                    e_pad = np.zeros(n, np.int64)
                    e_pad[:len(es)] = es
                    if len(es):
                        k = np.arange(len(es))
                        S[c, k % P, base + k // P, ed] = one
                        ST[c, ed, base + k // P, k % P] = one
                    idx_all[c, :, base * 8:(base + nch) * 8] = _wrap16(e_pad)
        ch += lw + hw

    # blob: per group [idx (nch*16B) | S (nch*128B) | ST (nch*128B)]
    blob_parts = [[] for _ in range(N_CORES)]
    off = 0
    for m in meta:
        c0, nch = m["ch0"], m["lw"] + m["hw"]
        m["blob_off"] = off
        m["nch"] = nch
        off += nch * (16 + 128 + 128)
        for c in range(N_CORES):
            blob_parts[c].append(idx_all[c, :, c0 * 8:(c0 + nch) * 8].view(np.uint8))
            blob_parts[c].append(S[c, :, c0:c0 + nch, :].reshape(P, nch * 128).view(np.uint8))
            blob_parts[c].append(ST[c, :, c0:c0 + nch, :].reshape(P, nch * 128).view(np.uint8))
    blob = np.stack([np.concatenate(blob_parts[c], axis=1) for c in range(N_CORES)])
    return dict(meta=meta, TOTB=off, TOTCH=TOTCH, blob=np.ascontiguousarray(blob),
                LCH=LCH, HCH=HCH)


def _rot_M(a_src, a_dst):
    """Per-head invertible M with columns [a_src | a_dst | complement]."""
    Hh, C = a_src.shape
    blocks = []
    for h in range(Hh):
        pair = np.stack([a_src[h], a_dst[h]], axis=1).astype(np.float64)
        Q, _ = np.linalg.qr(pair, mode="complete")
        M = np.concatenate([pair, Q[:, 2:]], axis=1)
        blocks.append(M)
    return blocks


def _blockdiag(blocks):
    n = sum(b.shape[0] for b in blocks)
    m = sum(b.shape[1] for b in blocks)
    out = np.zeros((n, m), np.float64)
    r = c = 0
    for b in blocks:
        out[r:r + b.shape[0], c:c + b.shape[1]] = b
        r += b.shape[0]
        c += b.shape[1]
    return out


# ------------------------------------------------------------- bass builders

class Prog:
    def __init__(self):
        self.nc = bacc.Bacc("TRN2", target_bir_lowering=False, debug=False,
                            num_devices=N_CORES,
                            dynamic_dma_scratch_size=32768)
        self.in_aps = {}
        self.out_aps = {}

    def inp(self, name, shape, dt):
        ap = self.nc.dram_tensor(name, list(shape), dt, kind="ExternalInput").ap()
        self.in_aps[name] = ap
        return ap

    def out(self, name, shape, dt):
        ap = self.nc.dram_tensor(name, list(shape), dt, kind="ExternalOutput").ap()
        self.out_aps[name] = ap
        return ap


def build_launch_A():
    """x_shard @ W1rot -> table1 rows (rotated h1, bf16)."""
    pr = Prog()
    nc = pr.nc
    x = pr.inp("x", [NSP, P], F32)
    w1 = pr.inp("w1", [P, F], F32)
    tab = pr.out("tab", [NSP, ROW], BF16)
    with tile.TileContext(nc) as tc, ExitStack() as ctx:
        sb = ctx.enter_context(tc.tile_pool(name="sb", bufs=5))
        ps = ctx.enter_context(tc.tile_pool(name="ps", bufs=4, space="PSUM"))
        cpool = ctx.enter_context(tc.tile_pool(name="cp", bufs=1))
        ident = cpool.tile([P, P], F32)
        make_identity(nc, ident[:])
        w1t = cpool.tile([P, F], F32)
        nc.sync.dma_start(w1t[:], w1)
        B4 = 7
        for t0 in range(0, NT, B4):
            nb = min(B4, NT - t0)
            xt = sb.tile([P, B4, P], F32, tag="xt")
            nc.sync.dma_start(
                xt[:, 0:nb, :],
                x[t0 * P:(t0 + nb) * P, :].rearrange("(b p) f -> p b f", p=P))
            trow = sb.tile([P, B4, ROW], BF16, tag="trow")
            for j in range(nb):
                xT_ps = ps.tile([P, P], F32, space="PSUM", tag="xT")
                nc.tensor.transpose(xT_ps[:], xt[:, j, :], ident[:])
                xT = sb.tile([P, P], F32, tag="xTs")
                nc.scalar.copy(xT[:], xT_ps[:])
                h_ps = ps.tile([P, F], F32, space="PSUM", tag="hps")
                nc.tensor.matmul(h_ps[:], lhsT=xT[:], rhs=w1t[:],
                                 start=True, stop=True)
                nc.scalar.copy(trow[:, j, :], h_ps[:])
            nc.scalar.dma_start(
                tab[t0 * P:(t0 + nb) * P, :].rearrange("(b p) f -> p b f", p=P),
                trow[:, 0:nb, :])
    nc.compile()
    return pr


def build_launch_agg(sch, layer, b3=0.0, inv_as3=1.0):
    """layer=1: L1 agg -> table2; layer=2: L2 agg -> table3; layer=3: out."""
    pr = Prog()
    nc = pr.nc
    last = layer == 3
    row = ROW3 if last else ROW
    nad = 1 if last else H
    table = pr.inp("table", [N_NODES, row], BF16)
    mytab = pr.inp("mytab", [NSP, row], BF16)
    blob_in = pr.inp("blob", [P, sch["TOTB"]], U8)
    if layer == 1:
        naug = F
        nwc = 8
        waug = pr.inp("waug", [F, naug], BF16)
        tabout = pr.out("tabout", [NSP, ROW], BF16)
    elif layer == 2:
        naug = 2
        nwc = 2
        waug = pr.inp("waug", [F, naug], BF16)
        tabout = pr.out("tabout", [NSP, ROW3], BF16)
    else:
        outv = pr.out("outv", [NSP, 1], F32)
    if not last:
        wc = pr.inp("wc", [P, nwc], F32)       # (ones@Waug)[slots], replicated
        minv = pr.inp("minv", [P, 2, P], BF16)  # per-half blockdiag(Minv)
        bias = pr.inp("bias", [P, 2], F32)
        nbias = pr.inp("nbias", [P, 2], F32)

    with tile.TileContext(nc) as tc, ExitStack() as ctx:
        sb = ctx.enter_context(tc.tile_pool(name="sb", bufs=2))
        sbg = ctx.enter_context(tc.tile_pool(name="sbg", bufs=2))
        ps = ctx.enter_context(tc.tile_pool(name="ps", bufs=2, space="PSUM"))
        psb = ctx.enter_context(tc.tile_pool(name="psb", bufs=1, space="PSUM"))
        # PSUM banks: combo(2) + agg(2) + yT(2) + xT(1) + hps(1) = 8
        cpool = ctx.enter_context(tc.tile_pool(name="cp", bufs=1))
        ident8 = cpool.tile([P, P], FP8)
        make_identity(nc, ident8[:])
        if not last:
            identb = cpool.tile([P, P], BF16)
            make_identity(nc, identb[:])
            waug_t = cpool.tile([P, F // P, naug], BF16, tag="waug")
            for k in range(F // P):
                nc.sync.dma_start(waug_t[:, k, :], waug[k * P:(k + 1) * P, :])
            wc_t = cpool.tile([P, nwc], F32, tag="wc")
            nc.sync.dma_start(wc_t[:], wc)
            minv_t = cpool.tile([P, 2, P], BF16, tag="minv")
            nc.sync.dma_start(minv_t[:], minv)
            bias_t = cpool.tile([P, 2], F32, tag="bias")
            nc.sync.dma_start(bias_t[:], bias)
            nbias_t = cpool.tile([P, 2], F32, tag="nbias")
            nc.sync.dma_start(nbias_t[:], nbias)
        else:
            obuf = cpool.tile([P, NT], F32, tag="obuf")

        for m in sch["meta"]:
            t0, gt, lw, hw, nch = m["t0"], m["gt"], m["lw"], m["hw"], m["nch"]
            boff = m["blob_off"]
            tinfo = m["tinfo"]
            # ---- loads
            blob_t = sbg.tile([P, nch * 272], U8, tag="blob")
            nc.sync.dma_start(blob_t[:], blob_in[:, boff:boff + nch * 272])
            idx_v = blob_t[:, 0:nch * 16].bitcast(I16)
            s_v = blob_t[:, nch * 16:nch * 144].bitcast(FP8).rearrange(
                "p (c e) -> p c e", e=P)
            st_v = blob_t[:, nch * 144:nch * 272].bitcast(FP8).rearrange(
                "p (c e) -> p c e", e=P)
            selfr = sb.tile([P, gt, row], BF16, tag="selfr")
            nc.sync.dma_start(
                selfr[:],
                mytab[t0 * P:(t0 + gt) * P, :].rearrange("(b p) f -> p b f", p=P))
            # ---- gathers
            g_lo = g_hi = None
            if lw:
                g_lo = sbg.tile([P, lw, row], BF16, tag="glo")
                for w0 in range(0, lw, GMAX):
                    w1_ = min(w0 + GMAX, lw)
                    nc.gpsimd.dma_gather(
                        out_ap=g_lo[:, w0:w1_, :], in_ap=table,
                        idxs_ap=idx_v[:, w0 * 8:w1_ * 8],
                        num_idxs=(w1_ - w0) * P, num_idxs_reg=(w1_ - w0) * P,
                        elem_size=row)
            if hw:
                g_hi = sbg.tile([P, hw, row], BF16, tag="ghi")
                for w0 in range(0, hw, GMAX):
                    w1_ = min(w0 + GMAX, hw)
                    nc.gpsimd.dma_gather(
                        out_ap=g_hi[:, w0:w1_, :], in_ap=table[HALF:, :],
                        idxs_ap=idx_v[:, (lw + w0) * 8:(lw + w1_) * 8],
                        num_idxs=(w1_ - w0) * P, num_idxs_reg=(w1_ - w0) * P,
                        elem_size=row)

            # ---- as/ad of own nodes (slots 0,1 per head)
            if not last:
                selfr_v = selfr[:].rearrange("p b (h c) -> p b h c", h=H)
                asad = sb.tile([P, gt, H, 2], BF16, tag="asad")
                nc.vector.tensor_copy(asad[:], selfr_v[:, :, :, 0:2])
                as_own, ad_own = asad[:, :, :, 0], asad[:, :, :, 1]
            else:
                as_own, ad_own = selfr[:, :, 0:1], selfr[:, :, 1:2]

            # ---- a_dst expansion (zps) per chunk; combo also holds
            # the r-expansion (zps2) and per-tile denominators in one bank
            combo = ps.tile([P, 2 * nch + gt, nad], F32, space="PSUM", tag="combo")
            zps = combo[:, 0:nch, :]
            zps2 = combo[:, nch:2 * nch, :]
            den = combo[:, 2 * nch:2 * nch + gt, :]
            for j in range(gt):
                lo0, lcnt, hi0, hcnt = tinfo[j]
                adt = sb.tile([P, nad], FP16, tag="adt")
                nc.vector.tensor_copy(adt[:], ad_own[:, j, :])
                for cj in (list(range(lo0, lo0 + lcnt))
                           + list(range(lw + hi0, lw + hi0 + hcnt))):
                    nc.tensor.matmul(zps[:, cj, :], lhsT=st_v[:, cj, :],
                                     rhs=adt[:], start=True, stop=True,
                                     skip_group_check=True)

            # ---- z, e (chunk cols then self cols)
            ncol = nch + gt
            z_t = sb.tile([P, ncol, nad], F32, tag="z")
            if lw:
                as_lo = g_lo[:].rearrange("p c (h f) -> p c h f", h=nad)[:, :, :, 0]
                nc.vector.tensor_tensor(out=z_t[:, 0:lw, :], in0=as_lo,
                                        in1=zps[:, 0:lw, :], op=mybir.AluOpType.add)
            if hw:
                as_hi = g_hi[:].rearrange("p c (h f) -> p c h f", h=nad)[:, :, :, 0]
                nc.vector.tensor_tensor(out=z_t[:, lw:nch, :], in0=as_hi,
                                        in1=zps[:, lw:nch, :], op=mybir.AluOpType.add)
            nc.vector.tensor_tensor(out=z_t[:, nch:ncol, :], in0=as_own,
                                    in1=ad_own, op=mybir.AluOpType.add)
            l_t = sb.tile([P, ncol, nad], F32, tag="l")
            nc.scalar.activation(l_t[:], z_t[:],
                                 mybir.ActivationFunctionType.Prelu,
                                 alpha=NEG_SLOPE)
            e_t = sb.tile([P, ncol, nad], BF16, tag="e")
            nc.scalar.activation(e_t[:], l_t[:],
                                 mybir.ActivationFunctionType.Exp)

            # ---- denominators per tile, then r = 1/(den+eps)
            for j in range(gt):
                lo0, lcnt, hi0, hcnt = tinfo[j]
                cjs = (list(range(lo0, lo0 + lcnt))
                       + list(range(lw + hi0, lw + hi0 + hcnt)))
                for i, cj in enumerate(cjs):
                    nc.tensor.matmul(den[:, j, :], lhsT=s_v[:, cj, :],
                                     rhs=e_t[:, cj, :], start=(i == 0), stop=False,
                                     skip_group_check=True)
                nc.tensor.matmul(den[:, j, :], lhsT=ident8[:],
                                 rhs=e_t[:, nch + j, :], start=False, stop=True,
                                 skip_group_check=True)
            r_t = sb.tile([P, gt, nad], F32, tag="r")
            nc.vector.tensor_scalar_add(r_t[:], den[:], 1e-16)
            nc.vector.reciprocal(r_t[:], r_t[:])
            r16 = sb.tile([P, gt, nad], FP16, tag="r16")
            nc.vector.tensor_copy(r16[:], r_t[:])

            # ---- alpha = e * expand(r)
            for j in range(gt):
                lo0, lcnt, hi0, hcnt = tinfo[j]
                for cj in (list(range(lo0, lo0 + lcnt))
                           + list(range(lw + hi0, lw + hi0 + hcnt))):
                    nc.tensor.matmul(zps2[:, cj, :], lhsT=st_v[:, cj, :],
                                     rhs=r16[:, j, :], start=True, stop=True,
                                     skip_group_check=True)
            alpha = sb.tile([P, ncol, nad], BF16, tag="alpha")
            nc.vector.tensor_tensor(out=alpha[:, 0:nch, :], in0=e_t[:, 0:nch, :],
                                    in1=zps2, op=mybir.AluOpType.mult)
            nc.vector.tensor_tensor(out=alpha[:, nch:ncol, :],
                                    in0=e_t[:, nch:ncol, :], in1=r_t[:],
                                    op=mybir.AluOpType.mult)

            # ---- weighted messages and aggregation
            nag = row if not last else 1
            agg = psb.tile([P, gt, nag], F32, space="PSUM", tag="agg")
            # (agg and yT live in the bufs=1 pool: 2 banks each)
            if not last:
                apair = sb.tile([P, ncol, nad, 1, 2], BF16, tag="apair")
                nc.vector.tensor_copy(
                    apair[:], alpha[:].broadcast_to([P, ncol, nad, 1, 2]))
            for j in range(gt):
                lo0, lcnt, hi0, hcnt = tinfo[j]
                nch_t = lcnt + hcnt
                eg = sb.tile([P, max(nch_t, 1), nag], BF16, tag="eg")
                eg_s = sb.tile([P, nag], BF16, tag="egs")
                if not last:
                    egv = eg[:].rearrange("p c (h r t) -> p c h r t", h=H, t=2)
                    if lcnt:
                        nc.vector.tensor_tensor(
                            out=egv[:, 0:lcnt],
                            in0=g_lo[:, lo0:lo0 + lcnt, :].rearrange(
                                "p c (h r t) -> p c h r t", h=H, t=2),
                            in1=apair[:, lo0:lo0 + lcnt].broadcast_to(
                                [P, lcnt, H, 32, 2]),
                            op=mybir.AluOpType.mult)
                    if hcnt:
                        nc.vector.tensor_tensor(
                            out=egv[:, lcnt:nch_t],
                            in0=g_hi[:, hi0:hi0 + hcnt, :].rearrange(
                                "p c (h r t) -> p c h r t", h=H, t=2),
                            in1=apair[:, lw + hi0:lw + hi0 + hcnt].broadcast_to(
                                [P, hcnt, H, 32, 2]),
                            op=mybir.AluOpType.mult)
                    nc.vector.tensor_tensor(
                        out=eg_s[:].rearrange("p (h r t) -> p h r t", h=H, t=2),
                        in0=selfr[:, j, :].rearrange("p (h r t) -> p h r t",
                                                     h=H, t=2),
                        in1=apair[:, nch + j].broadcast_to([P, H, 32, 2]),
                        op=mybir.AluOpType.mult)
                else:
                    if lcnt:
                        nc.vector.tensor_tensor(
                            out=eg[:, 0:lcnt, :], in0=g_lo[:, lo0:lo0 + lcnt, 0:1],
                            in1=alpha[:, lo0:lo0 + lcnt, :], op=mybir.AluOpType.mult)
                    if hcnt:
                        nc.vector.tensor_tensor(
                            out=eg[:, lcnt:nch_t, :],
                            in0=g_hi[:, hi0:hi0 + hcnt, 0:1],
                            in1=alpha[:, lw + hi0:lw + hi0 + hcnt, :],
                            op=mybir.AluOpType.mult)
                    nc.vector.tensor_tensor(
                        out=eg_s[:], in0=selfr[:, j, 0:1],
                        in1=alpha[:, nch + j, :], op=mybir.AluOpType.mult)
                cjs = (list(range(lo0, lo0 + lcnt))
                       + list(range(lw + hi0, lw + hi0 + hcnt)))
                for i, cj in enumerate(cjs):
                    nc.tensor.matmul(agg[:, j, :], lhsT=s_v[:, cj, :],
                                     rhs=eg[:, i, :], start=(i == 0), stop=False)
                nc.tensor.matmul(agg[:, j, :], lhsT=ident8[:],
                                 rhs=eg_s[:], start=(nch_t == 0), stop=True)

            # ---- epilogue
            if last:
                for j in range(gt):
                    t = t0 + j
                    nc.vector.tensor_scalar(
                        out=obuf[:, t:t + 1], in0=agg[:, j, :],
                        scalar1=float(inv_as3), scalar2=float(b3),
                        op0=mybir.AluOpType.mult, op1=mybir.AluOpType.add)
                continue

            yT = psb.tile([P, gt, 2, P], F32, space="PSUM", tag="yT")
            for j in range(gt):
                xn = sb.tile([P, F], BF16, tag="xn")
                nc.scalar.copy(xn[:], agg[:, j, :])
                for k in range(2):
                    xT_ps = psb.tile([P, P], BF16, space="PSUM", tag="xT")
                    nc.tensor.transpose(xT_ps[:], xn[:, k * P:(k + 1) * P],
                                        identb[:])
                    xTs = sb.tile([P, P], BF16, tag="xTs")
                    nc.scalar.copy(xTs[:], xT_ps[:])
                    nc.tensor.matmul(yT[:, j, k, :], lhsT=minv_t[:, k, :],
                                     rhs=xTs[:], start=True, stop=True)
            xe_g = sb.tile([P, gt, 2, P], BF16, tag="xe")
            for k in range(2):
                p_k = sb.tile([P, gt, P], BF16, tag="pk")
                nc.scalar.activation(p_k[:], yT[:, :, k, :],
                                     mybir.ActivationFunctionType.Relu,
                                     bias=bias_t[:, k:k + 1])
                m_k = sb.tile([P, gt, P], F32, tag="mk")
                nc.scalar.activation(m_k[:], yT[:, :, k, :],
                                     mybir.ActivationFunctionType.Relu,
                                     bias=nbias_t[:, k:k + 1], scale=-1.0)
                q_k = sb.tile([P, gt, P], BF16, tag="qk")
                nc.scalar.activation(q_k[:], m_k[:],
                                     mybir.ActivationFunctionType.Exp,
                                     scale=-1.0)
                nc.vector.tensor_tensor(out=xe_g[:, :, k, :], in0=p_k[:],
                                        in1=q_k[:], op=mybir.AluOpType.add)
            trow = sb.tile([P, gt, ROW if layer == 1 else ROW3], BF16, tag="trow")
            if layer == 2:
                nc.vector.memset(trow[:], 0.0)
            for j in range(gt):
                h_ps = psb.tile([P, naug], F32, space="PSUM", tag="hps")
                for k in range(2):
                    nc.tensor.matmul(h_ps[:], lhsT=xe_g[:, j, k, :],
                                     rhs=waug_t[:, k, :], start=(k == 0),
                                     stop=(k == 1))
                if layer == 1:
                    nc.scalar.copy(trow[:, j, :], h_ps[:])
                    # overwrite as/ad slots with the exact (-1-corrected) values
                    nc.vector.tensor_tensor(
                        out=trow[:, j, :].rearrange("p (h c) -> p h c",
                                                    h=H)[:, :, 0:2],
                        in0=h_ps[:].rearrange("p (h c) -> p h c", h=H)[:, :, 0:2],
                        in1=wc_t[:].rearrange("p (h c) -> p h c", h=H),
                        op=mybir.AluOpType.subtract)
                else:
                    nc.vector.tensor_tensor(
                        out=trow[:, j, 0:2], in0=h_ps[:], in1=wc_t[:],
                        op=mybir.AluOpType.subtract)
            nc.scalar.dma_start(
                tabout[t0 * P:(t0 + gt) * P, :].rearrange("(b p) f -> p b f", p=P),
                trow[:])
        if last:
            nc.scalar.dma_start(
                outv.rearrange("(t p) o -> p t o", p=P).squeeze(-1), obuf[:])
    nc.compile()
    return pr


# --------------------------------------------------------------- the kernel

LAST_TIMES = {}


def _run(pr, in_maps, tag=None):
    if tag is not None:
        try:
            from concourse.timeline_sim import TimelineSim
            LAST_TIMES[tag] = TimelineSim(pr.nc, trace=False).simulate() / 1e9
        except Exception:
            pass
    res = bass_utils.run_bass_kernel_spmd(
        pr.nc, in_maps, core_ids=list(range(N_CORES)))
    return res.results


def _pad_rows(a, n):
    out = np.zeros((n,) + a.shape[1:], a.dtype)
    out[:len(a)] = a
    return out


def kernel(x, edge_index, W1, a_src1, a_dst1, b1, W2, a_src2, a_dst2, b2,
           W3, a_src3, a_dst3, b3):
    x = np.asarray(x, np.float32)
    ei = np.asarray(edge_index)
    src = ei[0].astype(np.int64)
    dst = ei[1].astype(np.int64)

    pos = _balance_perm(dst)
    node_of_pos = np.empty(N_NODES, np.int64)
    node_of_pos[pos] = np.arange(N_NODES)
    sch = build_schedule(src, dst, pos)

    W1 = np.asarray(W1, np.float64)
    W2 = np.asarray(W2, np.float64)
    W3 = np.asarray(W3, np.float64)
    M1 = _rot_M(np.asarray(a_src1), np.asarray(a_dst1))
    M2 = _rot_M(np.asarray(a_src2), np.asarray(a_dst2))
    BD1, BD2 = _blockdiag(M1), _blockdiag(M2)
    W1rot = (W1 @ BD1).astype(np.float32)
    W2rot = (W2 @ BD2).astype(np.float32)
    a_s3 = float(np.asarray(a_src3).reshape(-1)[0])
    a_d3 = float(np.asarray(a_dst3).reshape(-1)[0])
    W3aug = np.concatenate([W3 * a_s3, W3 * a_d3], 1).astype(np.float32)

    Minv1 = _blockdiag([np.linalg.inv(m) for m in M1])
    Minv2 = _blockdiag([np.linalg.inv(m) for m in M2])
    minv1_t = np.stack([Minv1[k * P:(k + 1) * P, k * P:(k + 1) * P]
                        for k in range(2)]).transpose(1, 0, 2)
    minv2_t = np.stack([Minv2[k * P:(k + 1) * P, k * P:(k + 1) * P]
                        for k in range(2)]).transpose(1, 0, 2)

    w2row = W2rot.sum(0).astype(np.float64)            # ones @ W2rot
    w3row = W3aug.sum(0).astype(np.float64)
    wc2 = np.tile(w2row[SLOTS01].astype(np.float32), (P, 1))
    wc3 = np.tile(w3row.astype(np.float32), (P, 1))
    w2m = w2row.copy()
    w2m[SLOTS01] = 0.0
    b1v = np.asarray(b1, np.float64)
    b2v = np.asarray(b2, np.float64)
    b1_eff = b1v
    b2_eff = b2v - (w2m @ Minv2)
    b1T = b1_eff.astype(np.float32).reshape(2, P).T.copy()
    b2T = b2_eff.astype(np.float32).reshape(2, P).T.copy()

    bf = ml_dtypes.bfloat16
    consts1 = dict(waug=W2rot.astype(bf), wc=wc2, minv=minv1_t.astype(bf),
                   bias=b1T, nbias=np.ascontiguousarray(-b1T))
    consts2 = dict(waug=W3aug.astype(bf), wc=wc3, minv=minv2_t.astype(bf),
                   bias=b2T, nbias=np.ascontiguousarray(-b2T))

    xp = x[node_of_pos]                                 # permuted rows

    prA = build_launch_A()
    inA = [dict(x=_pad_rows(xp[c * NS:(c + 1) * NS], NSP), w1=W1rot)
           for c in range(N_CORES)]
    resA = _run(prA, inA, tag="A")
    tab1 = np.ascontiguousarray(
        np.concatenate([resA[c]["tab"][:NS] for c in range(N_CORES)], 0))

    prB = build_launch_agg(sch, 1)
    inB = [dict(table=tab1, mytab=_pad_rows(tab1[c * NS:(c + 1) * NS], NSP),
                blob=sch["blob"][c], **consts1) for c in range(N_CORES)]
    resB = _run(prB, inB, tag="B")
    tab2 = np.ascontiguousarray(
        np.concatenate([resB[c]["tabout"][:NS] for c in range(N_CORES)], 0))

    prC = build_launch_agg(sch, 2)
    inC = [dict(table=tab2, mytab=_pad_rows(tab2[c * NS:(c + 1) * NS], NSP),
                blob=sch["blob"][c], **consts2) for c in range(N_CORES)]
    resC = _run(prC, inC, tag="C")
    tab3 = np.ascontiguousarray(
        np.concatenate([resC[c]["tabout"][:NS] for c in range(N_CORES)], 0))

    prD = build_launch_agg(sch, 3, b3=float(np.asarray(b3).reshape(-1)[0]),
                           inv_as3=1.0 / a_s3)
    inD = [dict(table=tab3, mytab=_pad_rows(tab3[c * NS:(c + 1) * NS], NSP),
                blob=sch["blob"][c]) for c in range(N_CORES)]
    resD = _run(prD, inD, tag="D")
    outp = np.concatenate([resD[c]["outv"][:NS] for c in range(N_CORES)], 0)
    out = outp[pos]                                     # back to node order
    return np.ascontiguousarray(out.astype(np.float32))
